# revision 15
# baseline (speedup 1.0000x reference)
import sys, os
sys.path.insert(0, "/opt/trn_rl_repo")
sys.path.insert(0, "/opt/trn_rl_repo/concourse")

import numpy as np

T, HW, M = 16, 1024, 512
D_MODEL, D_K, H = 1024, 512, 8
HD = D_K // H      # 64
VD = D_MODEL // H  # 128
SIGMA = 0.5
EPS = 1e-6
NCORES = 8
F = T // NCORES    # frames per core = 2
NB = 10            # split-precision bias rows

_F32 = np.float32

# per-head rope de-interleave permutation of the D_K dims:
# head h keeps cols [64h,64h+64); within a head evens first then odds.
_PI = np.empty(D_K, np.int64)
for _h in range(H):
    for _i in range(32):
        _PI[64 * _h + _i] = 64 * _h + 2 * _i
        _PI[64 * _h + 32 + _i] = 64 * _h + 2 * _i + 1


def _bf16(x):
    import ml_dtypes
    return np.asarray(x, np.float32).astype(ml_dtypes.bfloat16)


def _r11(x):
    """Round fp32 to 10 explicit mantissa bits (f32r/tf32-representable)."""
    b = np.asarray(x, np.float32).view(np.uint32)
    b = (b + 0x1000) & np.uint32(0xFFFFE000)
    return b.view(np.float32)


def _host_prep(updated_track_tokens, tracks, feature_positions, original_features,
               grid_coords_tokens, W_q, W_k, W_out, q_gamma, k_gamma):
    utt = np.ascontiguousarray(updated_track_tokens, dtype=_F32)      # [T, M, D]
    grid = np.ascontiguousarray(grid_coords_tokens, dtype=_F32)       # [T, HW, D]
    trk = np.ascontiguousarray(tracks, dtype=_F32)                    # [T, M, 2]
    fp = np.ascontiguousarray(feature_positions, dtype=_F32)          # [HW, 2]

    gridT = _bf16(grid.transpose(0, 2, 1))                            # [T, D, HW]
    uttT = _bf16(utt.transpose(0, 2, 1))                              # [T, D, M]

    # V = rope_2d(utt, tracks) computed on host (elementwise table math)
    def rope_2d(x, pos):
        B, N, D = x.shape
        half, quarter = D // 2, D // 4
        theta = 1.0 / (10000.0 ** (2.0 * np.arange(quarter, dtype=_F32) / half))
        fx = pos[..., 0:1] * theta
        fy = pos[..., 1:2] * theta

        def rot(part, f):
            c, s = np.cos(f).astype(_F32), np.sin(f).astype(_F32)
            p = part.reshape(B, N, quarter, 2)
            x1, x2 = p[..., 0], p[..., 1]
            return np.stack([x1 * c - x2 * s, x1 * s + x2 * c], -1).reshape(B, N, half)

        return np.concatenate([rot(x[..., :half], fx), rot(x[..., half:], fy)], -1)

    vro = _bf16(rope_2d(utt, trk))                                    # [T, M, D]

    # K-rope trig tables matching pi layout: [T, M, 512] = cos|cos|sin|sin blocks
    quarter = D_K // 4  # 128
    theta = (1.0 / (10000.0 ** (2.0 * np.arange(quarter, dtype=_F32) / (D_K // 2)))).astype(_F32)
    fx = trk[..., 0:1] * theta                                        # [T, M, 128]
    fy = trk[..., 1:2] * theta
    tct = _bf16(np.concatenate([np.cos(fx), np.cos(fy),
                                np.sin(fx), np.sin(fy)], axis=-1))    # [T, M, 512]

    # Split-precision bias rows: sum_r Bk[r,k] * Bq[r,q] == -16*|t_k - f_q|^2
    # with every large product exactly representable under tf32-grade rounding
    # and the residual products bounded small.
    th = (np.round(trk * 16.0) / 16.0).astype(_F32)                   # coarse 9-bit
    tl = (trk - th).astype(_F32)
    fh = (np.round(fp * 16.0) / 16.0).astype(_F32)
    fl = (fp - fh).astype(_F32)
    t2 = -16.0 * (trk.astype(np.float64) ** 2).sum(-1)                # [T, M]
    t2h = _r11(t2)
    t2l = (t2 - t2h.astype(np.float64)).astype(_F32)
    f2 = -16.0 * (fp.astype(np.float64) ** 2).sum(-1)                 # [HW]
    f2h = _r11(f2)
    f2l = (f2 - f2h.astype(np.float64)).astype(_F32)

    onesM = np.ones((T, M), _F32)
    bk = np.stack([32.0 * th[..., 0], 32.0 * th[..., 1],
                   32.0 * th[..., 0], 32.0 * th[..., 1],
                   32.0 * tl[..., 0], 32.0 * tl[..., 1],
                   t2h, t2l, onesM, onesM], axis=1).astype(_F32)      # [T, 10, M]
    onesQ = np.ones(HW, _F32)
    bq = np.stack([fh[:, 0], fh[:, 1], fl[:, 0], fl[:, 1],
                   fp[:, 0], fp[:, 1],
                   onesQ, onesQ, f2h, f2l], axis=0).astype(_F32)      # [10, HW]

    wq = _bf16(np.asarray(W_q, _F32)[:, _PI])
    wk = _bf16(np.asarray(W_k, _F32)[:, _PI])
    wout = _bf16(W_out)
    gam = (np.asarray(q_gamma, _F32) * np.asarray(k_gamma, _F32))[_PI]
    kgam = _bf16(np.broadcast_to(gam, (128, D_K)))
    ident = _bf16(np.eye(128, dtype=_F32))
    ones128 = _bf16(np.ones((128, 128), dtype=_F32))
    return dict(gridT=gridT, uttT=uttT, vro=vro, tct=tct, bk=bk, bq=bq,
                wq=wq, wk=wk, wout=wout, kgam=kgam, ident=ident, ones=ones128)


def _build_nc():
    import concourse.bass as bass
    import concourse.bacc as bacc
    from concourse import mybir
    from concourse import tile
    from contextlib import ExitStack

    f32 = mybir.dt.float32
    f32r = mybir.dt.float32r
    bf16 = mybir.dt.bfloat16
    MUL = mybir.AluOpType.mult
    SUB = mybir.AluOpType.subtract
    ADD = mybir.AluOpType.add
    AF = mybir.ActivationFunctionType

    nc = bacc.Bacc(None, target_bir_lowering=False, debug=False)

    gridT_d = nc.declare_dram_parameter("gridT", [F, D_MODEL, HW], bf16, False)
    uttT_d = nc.declare_dram_parameter("uttT", [F, D_MODEL, M], bf16, False)
    vro_d = nc.declare_dram_parameter("vro", [F, M, D_MODEL], bf16, False)
    tct_d = nc.declare_dram_parameter("tct", [F, M, D_K], bf16, False)
    bk_d = nc.declare_dram_parameter("bk", [F, NB, M], f32r, False)
    bq_d = nc.declare_dram_parameter("bq", [NB, HW], f32r, False)
    wq_d = nc.declare_dram_parameter("wq", [D_MODEL, D_K], bf16, False)
    wk_d = nc.declare_dram_parameter("wk", [D_MODEL, D_K], bf16, False)
    wout_d = nc.declare_dram_parameter("wout", [D_MODEL, D_MODEL], bf16, False)
    kgam_d = nc.declare_dram_parameter("kgam", [128, D_K], bf16, False)
    id_d = nc.declare_dram_parameter("ident", [128, 128], bf16, False)
    ones_d = nc.declare_dram_parameter("ones", [128, 128], bf16, False)
    out_d = nc.declare_dram_parameter("out", [F, HW, D_MODEL], bf16, True)

    with tile.TileContext(nc) as tc_:
        with ExitStack() as stack:
            ep = stack.enter_context
            pconst = ep(tc_.tile_pool(name="pconst", bufs=1))
            pw = ep(tc_.tile_pool(name="pw", bufs=1))
            pwout = ep(tc_.tile_pool(name="pwout", bufs=1))
            pqt = ep(tc_.tile_pool(name="pqt", bufs=1))
            pkt = ep(tc_.tile_pool(name="pkt", bufs=1))
            pv = ep(tc_.tile_pool(name="pv", bufs=1))
            pnt = ep(tc_.tile_pool(name="pnt", bufs=1))
            pbias = ep(tc_.tile_pool(name="pbias", bufs=2))
            ppex = ep(tc_.tile_pool(name="ppex", bufs=8))
            pinu = ep(tc_.tile_pool(name="pinu", bufs=1))
            ping = ep(tc_.tile_pool(name="ping", bufs=2))
            ptrig = ep(tc_.tile_pool(name="ptrig", bufs=4))
            pkro = ep(tc_.tile_pool(name="pkro", bufs=2))
            prt = ep(tc_.tile_pool(name="prt", bufs=1))
            pkn = ep(tc_.tile_pool(name="pkn", bufs=3))
            pst = ep(tc_.tile_pool(name="pst", bufs=4))
            prcp = ep(tc_.tile_pool(name="prcp", bufs=2))
            pf = ep(tc_.tile_pool(name="pf", bufs=6))
            poo = ep(tc_.tile_pool(name="poo", bufs=4))
            ps_acc = ep(tc_.tile_pool(name="ps_acc", bufs=5, space="PSUM"))
            ps_tp = ep(tc_.tile_pool(name="ps_tp", bufs=2, space="PSUM"))
            ps_dn = ep(tc_.tile_pool(name="ps_dn", bufs=1, space="PSUM"))

            ident_s = pconst.tile([128, 128], bf16, tag="ident")
            nc.sync.dma_start(ident_s[:], id_d[:])
            ones_s = pconst.tile([128, 128], bf16, tag="ones")
            nc.sync.dma_start(ones_s[:], ones_d[:])
            kgam_s = pconst.tile([128, D_K], bf16, tag="kgam")
            nc.sync.dma_start(kgam_s[:], kgam_d[:])
            # bias Q-side rows, duplicated at partitions 0 and 64
            bq_s = pconst.tile([128, HW], f32r, tag="bq")
            nc.sync.dma_start(bq_s[0:NB, :], bq_d[:])
            nc.sync.dma_start(bq_s[64:64 + NB, :], bq_d[:])
            eps_s = pconst.tile([128, 1], f32, tag="eps")
            nc.vector.memset(eps_s[:], EPS)

            wk_all = pw.tile([128, 8 * D_K], bf16, tag="wk_all")
            nc.sync.dma_start(wk_all[:].rearrange("p (kc j) -> p kc j", kc=8),
                              wk_d[:].rearrange("(kc p) j -> p kc j", kc=8))
            wq_all = pw.tile([128, 8 * D_K], bf16, tag="wq_all")

            def load_wq():
                nc.sync.dma_start(wq_all[:].rearrange("p (kc j) -> p kc j", kc=8),
                                  wq_d[:].rearrange("(kc p) j -> p kc j", kc=8))
            wq_s = [wq_all[:, kc * D_K:(kc + 1) * D_K] for kc in range(8)]
            wk_s = [wk_all[:, kc * D_K:(kc + 1) * D_K] for kc in range(8)]

            wout_all = pwout.tile([128, 8 * D_MODEL], bf16, tag="wout_all")
            wout_s = {(h, nb): wout_all[:, (2 * h + nb) * 512:(2 * h + nb + 1) * 512]
                      for h in range(8) for nb in range(2)}

            def load_wout():
                nc.sync.dma_start(
                    wout_all[:].rearrange("p (h j) -> p h j", h=8),
                    wout_d[:].rearrange("(h p) j -> p h j", h=8))

            def ln_stats(src_ap):
                """-> (rstd, negmurstd) [128,1] f32 tiles"""
                st6 = pst.tile([128, 6], f32, tag="st6")
                nc.vector.bn_stats(st6[:], src_ap)
                agg = pst.tile([128, 2], f32, tag="agg")
                nc.vector.bn_aggr(agg[:], st6[:])
                sd = pst.tile([128, 1], f32, tag="sd")
                nc.scalar.activation(sd[:], agg[:, 1:2], AF.Sqrt, bias=eps_s[:], scale=1.0)
                rstd = pst.tile([128, 1], f32, tag="rstd")
                nc.vector.reciprocal(rstd[:], sd[:])
                nmr = pst.tile([128, 1], f32, tag="nmr")
                nc.vector.scalar_tensor_tensor(nmr[:], agg[:, 0:1], -1.0, rstd[:],
                                               op0=MUL, op1=MUL)
                return rstd, nmr

            for f in range(F):
                # ---------- per-frame prefetches ----------
                Vro = [pv.tile([128, D_MODEL], bf16, tag=f"V{i}", name=f"V{i}") for i in range(4)]
                trig = []
                for mb in range(4):
                    tt = ptrig.tile([128, 512], bf16, tag="tct", name=f"tct{mb}")
                    nc.sync.dma_start(tt[:], tct_d[f, mb * 128:(mb + 1) * 128, :])
                    trig.append((tt[:, 0:256], tt[:, 256:512]))
                bk_s = pbias.tile([128, M], f32r, tag="bk")
                nc.sync.dma_start(bk_s[0:NB, :], bk_d[f])
                nc.sync.dma_start(bk_s[64:64 + NB, :], bk_d[f])

                # ---------- Phase B: K = gamma' * LN(rope(utt @ Wk)) -> KT2 ----------
                KT2 = [pkt.tile([128, M], bf16, tag=f"KT{i}_{f}", name=f"KT{i}_{f}") for i in range(4)]
                ut_all = pinu.tile([128, 8 * M], bf16, tag="ut")
                nc.sync.dma_start(ut_all[:].rearrange("p (kc m) -> p kc m", kc=8),
                                  uttT_d[f].rearrange("(kc p) m -> p kc m", kc=8))
                if f == 0:
                    load_wq()
                gts = []
                for qh in range(2):
                    g = ping.tile([128, 8 * 512], bf16, tag="gt", name=f"gt{qh}")
                    nc.sync.dma_start(
                        g[:].rearrange("p (kc q) -> p kc q", kc=8),
                        gridT_d[f, :, qh * 512:(qh + 1) * 512].rearrange(
                            "(kc p) q -> p kc q", kc=8))
                    gts.append(g)
                k_ps = {}
                for mbp in range(2):
                    for mb in (2 * mbp, 2 * mbp + 1):
                        k_ps[mb] = ps_acc.tile([128, D_K], f32, tag="acc", name=f"kps{mb}")
                    for kc in range(8):
                        ut = ut_all[:, kc * M:(kc + 1) * M]
                        for mb in (2 * mbp, 2 * mbp + 1):
                            nc.tensor.matmul(k_ps[mb][:], ut[:, mb * 128:(mb + 1) * 128],
                                             wk_s[kc], start=(kc == 0), stop=(kc == 7))
                for mb in range(4):
                    nc.sync.dma_start(Vro[mb][:], vro_d[f, mb * 128:(mb + 1) * 128, :])
                if f == 0:
                    load_wout()
                kf = []
                for mb in range(4):
                    t = pf.tile([128, D_K], f32, tag="pf", name=f"kf{mb}")
                    if mb % 2 == 0:
                        nc.vector.tensor_copy(t[:], k_ps[mb][:])
                    else:
                        nc.scalar.copy(t[:], k_ps[mb][:])
                    kf.append(t)
                for mb in range(4):
                    tcs, tss = trig[mb]
                    kro = pkro.tile([128, D_K], f32, tag="kro")
                    src3 = kf[mb][:].rearrange("p (h t) -> p h t", h=8)
                    dst3 = kro[:].rearrange("p (h t) -> p h t", h=8)
                    x1, x2 = src3[:, :, 0:32], src3[:, :, 32:64]
                    o1, o2 = dst3[:, :, 0:32], dst3[:, :, 32:64]
                    c3 = tcs.rearrange("p (h t) -> p h t", h=8)
                    s3 = tss.rearrange("p (h t) -> p h t", h=8)
                    t1 = prt.tile([128, 256], f32, tag="t1")
                    t2 = prt.tile([128, 256], f32, tag="t2")
                    t3 = prt.tile([128, 256], f32, tag="t3")
                    t4 = prt.tile([128, 256], f32, tag="t4")
                    t13 = t1[:].rearrange("p (h t) -> p h t", h=8)
                    t23 = t2[:].rearrange("p (h t) -> p h t", h=8)
                    t33 = t3[:].rearrange("p (h t) -> p h t", h=8)
                    t43 = t4[:].rearrange("p (h t) -> p h t", h=8)
                    nc.vector.tensor_tensor(t13, x1, c3, op=MUL)
                    nc.vector.tensor_tensor(t23, x2, s3, op=MUL)
                    nc.vector.tensor_tensor(o1, t13, t23, op=SUB)
                    nc.vector.tensor_tensor(t33, x1, s3, op=MUL)
                    nc.vector.tensor_tensor(t43, x2, c3, op=MUL)
                    nc.vector.tensor_tensor(o2, t33, t43, op=ADD)
                    rstd, nmr = ln_stats(kro[:])
                    kn0 = pkn.tile([128, D_K], bf16, tag="kn0")
                    nc.scalar.activation(kn0[:], kro[:], AF.Identity, bias=nmr[:], scale=rstd[:])
                    kn = pkn.tile([128, D_K], bf16, tag="kn")
                    nc.vector.tensor_tensor(kn[:], kn0[:], kgam_s[:], op=MUL)
                    tpt = ps_tp.tile([128, 1024], bf16, tag="tp")
                    for dc in range(4):
                        nc.tensor.transpose(tpt[:, dc * 128:(dc + 1) * 128],
                                            kn[:, dc * 128:(dc + 1) * 128], ident_s[:])
                    for dc in range(4):
                        if dc % 2 == 0:
                            nc.vector.tensor_copy(KT2[dc][:, mb * 128:(mb + 1) * 128],
                                                  tpt[:, dc * 128:(dc + 1) * 128])
                        else:
                            nc.scalar.copy(KT2[dc][:, mb * 128:(mb + 1) * 128],
                                           tpt[:, dc * 128:(dc + 1) * 128])

                # ---------- Phase A: Q = LN(grid @ Wq) -> QT2 ----------
                QT2 = [pqt.tile([128, HW], bf16, tag=f"QT{i}_{f}", name=f"QT{i}_{f}") for i in range(4)]
                for qh in range(2):
                    gt_all = gts[qh]
                    q_ps = {}
                    for qjp in range(2):
                        for qj in (2 * qjp, 2 * qjp + 1):
                            q_ps[qj] = ps_acc.tile([128, D_K], f32, tag="acc", name=f"qps{qj}")
                        for kc in range(8):
                            gt = gt_all[:, kc * 512:(kc + 1) * 512]
                            for qj in (2 * qjp, 2 * qjp + 1):
                                nc.tensor.matmul(q_ps[qj][:], gt[:, qj * 128:(qj + 1) * 128],
                                                 wq_s[kc], start=(kc == 0), stop=(kc == 7))
                    qf = []
                    for qj in range(4):
                        t = pf.tile([128, D_K], f32, tag="pf", name=f"qf{qj}")
                        if qj % 2 == 0:
                            nc.scalar.copy(t[:], q_ps[qj][:])
                        else:
                            nc.vector.tensor_copy(t[:], q_ps[qj][:])
                        qf.append(t)
                    for qj in range(4):
                        qb = qh * 4 + qj
                        rstd, nmr = ln_stats(qf[qj][:])
                        qn = pkn.tile([128, D_K], bf16, tag="qn")
                        nc.scalar.activation(qn[:], qf[qj][:], AF.Identity, bias=nmr[:], scale=rstd[:])
                        tpt = ps_tp.tile([128, 1024], bf16, tag="tp")
                        for dc in range(4):
                            nc.tensor.transpose(tpt[:, dc * 128:(dc + 1) * 128],
                                                qn[:, dc * 128:(dc + 1) * 128], ident_s[:])
                        for dc in range(4):
                            if dc % 2 == 0:
                                nc.scalar.copy(QT2[dc][:, qb * 128:(qb + 1) * 128],
                                               tpt[:, dc * 128:(dc + 1) * 128])
                            else:
                                nc.vector.tensor_copy(QT2[dc][:, qb * 128:(qb + 1) * 128],
                                                      tpt[:, dc * 128:(dc + 1) * 128])

                # ---------- Phase D: per-head attention (bias via f32r row-group mm) ----------
                NT = [pnt.tile([128, HW], bf16, tag=f"NT{i}", name=f"NT{i}") for i in range(8)]
                for h in range(8):
                    dc, sub = divmod(h, 2)
                    po = sub * 64
                    bo = 64 - po  # bias rows live in the other row-group half
                    Pex = [ppex.tile([128, HW], bf16, tag="pex", name=f"pex{i}") for i in range(4)]
                    for mb in range(4):
                        for nb in range(2):
                            sl = slice(nb * 512, (nb + 1) * 512)
                            s_ps = ps_acc.tile([128, 512], f32, tag="acc")
                            nc.tensor.matmul(
                                s_ps[:],
                                bk_s[bo:bo + NB, mb * 128:(mb + 1) * 128],
                                bq_s[bo:bo + NB, sl],
                                start=True, stop=False, tile_position=(bo, 0))
                            nc.tensor.matmul(
                                s_ps[:],
                                KT2[dc][po:po + 64, mb * 128:(mb + 1) * 128],
                                QT2[dc][po:po + 64, sl],
                                start=False, stop=True, tile_position=(po, 0))
                            nc.scalar.activation(Pex[mb][:, sl], s_ps[:],
                                                 AF.Exp, bias=0.0, scale=0.125)
                    for nb in range(2):
                        sl = slice(nb * 512, (nb + 1) * 512)
                        dnb = ps_dn.tile([128, 512], f32, tag="dn")
                        for mb in range(4):
                            nc.tensor.matmul(
                                dnb[:],
                                ones_s[:],
                                Pex[mb][:, sl],
                                start=(mb == 0), stop=(mb == 3))
                        rcp = prcp.tile([128, 512], f32, tag="rcp")
                        nc.vector.reciprocal_approx_fast(rcp[:], dnb[:])
                        nm_ps = ps_acc.tile([128, 512], f32, tag="acc")
                        for mb in range(4):
                            nc.tensor.matmul(
                                nm_ps[:],
                                Vro[mb][:, h * 128:(h + 1) * 128],
                                Pex[mb][:, sl],
                                start=(mb == 0), stop=(mb == 3))
                        nc.vector.tensor_tensor(
                            NT[h][:, sl], nm_ps[:], rcp[:], op=MUL)

                # ---------- Phase E: out = sampled @ Wout ----------
                for qb in range(8):
                    oo = poo.tile([128, 1024], bf16, tag="oo")
                    for nb in range(2):
                        o_ps = ps_acc.tile([128, 512], f32, tag="acc")
                        for h in range(8):
                            nc.tensor.matmul(
                                o_ps[:],
                                NT[h][:, qb * 128:(qb + 1) * 128],
                                wout_s[(h, nb)],
                                start=(h == 0), stop=(h == 7))
                        if nb == 0:
                            nc.vector.tensor_copy(oo[:, 0:512], o_ps[:])
                        else:
                            nc.scalar.copy(oo[:, 512:1024], o_ps[:])
                    nc.sync.dma_start(out_d[f, qb * 128:(qb + 1) * 128, :], oo[:])

    nc.compile()
    return nc


_NC_CACHE = None
_LAST = None


def _get_nc():
    global _NC_CACHE
    if _NC_CACHE is None:
        _NC_CACHE = _build_nc()
    return _NC_CACHE


def kernel(**inputs) -> np.ndarray:
    global _LAST
    prep = _host_prep(**inputs)
    from concourse.bass_utils import run_bass_kernel_spmd
    nc = _get_nc()
    in_maps = []
    for c in range(NCORES):
        sl = slice(c * F, (c + 1) * F)
        in_maps.append({
            "gridT": prep["gridT"][sl], "uttT": prep["uttT"][sl],
            "vro": prep["vro"][sl], "tct": prep["tct"][sl],
            "bk": prep["bk"][sl], "bq": prep["bq"],
            "wq": prep["wq"], "wk": prep["wk"], "wout": prep["wout"],
            "kgam": prep["kgam"], "ident": prep["ident"], "ones": prep["ones"],
        })
    trace = bool(os.environ.get("KBENCH_TRACE"))
    res = run_bass_kernel_spmd(nc, in_maps, core_ids=list(range(NCORES)),
                               trace=trace)
    _LAST = res
    out = np.concatenate([np.asarray(res.results[c]["out"]).astype(np.float32)
                          for c in range(NCORES)], axis=0)
    return np.ascontiguousarray(out.reshape(T, HW, D_MODEL))


# revision 16
# speedup vs baseline: 1.1061x; 1.1061x over previous
import sys, os
sys.path.insert(0, "/opt/trn_rl_repo")
sys.path.insert(0, "/opt/trn_rl_repo/concourse")

import numpy as np

T, HW, M = 16, 1024, 512
D_MODEL, D_K, H = 1024, 512, 8
HD = D_K // H      # 64
VD = D_MODEL // H  # 128
SIGMA = 0.5
EPS = 1e-6
NCORES = 8
F = T // NCORES    # frames per core = 2
NB = 18            # split-precision bias rows (bf16-exact 3-level split)

_F32 = np.float32

# per-head rope de-interleave permutation of the D_K dims:
# head h keeps cols [64h,64h+64); within a head evens first then odds.
_PI = np.empty(D_K, np.int64)
for _h in range(H):
    for _i in range(32):
        _PI[64 * _h + _i] = 64 * _h + 2 * _i
        _PI[64 * _h + 32 + _i] = 64 * _h + 2 * _i + 1


def _bf16(x):
    import ml_dtypes
    return np.asarray(x, np.float32).astype(ml_dtypes.bfloat16)


def _r11(x):
    """Round fp32 to 10 explicit mantissa bits (f32r/tf32-representable)."""
    b = np.asarray(x, np.float32).view(np.uint32)
    b = (b + 0x1000) & np.uint32(0xFFFFE000)
    return b.view(np.float32)


def _host_prep(updated_track_tokens, tracks, feature_positions, original_features,
               grid_coords_tokens, W_q, W_k, W_out, q_gamma, k_gamma):
    utt = np.ascontiguousarray(updated_track_tokens, dtype=_F32)      # [T, M, D]
    grid = np.ascontiguousarray(grid_coords_tokens, dtype=_F32)       # [T, HW, D]
    trk = np.ascontiguousarray(tracks, dtype=_F32)                    # [T, M, 2]
    fp = np.ascontiguousarray(feature_positions, dtype=_F32)          # [HW, 2]

    gridT = _bf16(grid.transpose(0, 2, 1))                            # [T, D, HW]
    uttT = _bf16(utt.transpose(0, 2, 1))                              # [T, D, M]

    # V = rope_2d(utt, tracks) computed on host (elementwise table math)
    def rope_2d(x, pos):
        B, N, D = x.shape
        half, quarter = D // 2, D // 4
        theta = 1.0 / (10000.0 ** (2.0 * np.arange(quarter, dtype=_F32) / half))
        fx = pos[..., 0:1] * theta
        fy = pos[..., 1:2] * theta

        def rot(part, f):
            c, s = np.cos(f).astype(_F32), np.sin(f).astype(_F32)
            p = part.reshape(B, N, quarter, 2)
            x1, x2 = p[..., 0], p[..., 1]
            return np.stack([x1 * c - x2 * s, x1 * s + x2 * c], -1).reshape(B, N, half)

        return np.concatenate([rot(x[..., :half], fx), rot(x[..., half:], fy)], -1)

    vro = _bf16(rope_2d(utt, trk))                                    # [T, M, D]

    # K-rope trig tables matching pi layout: [T, M, 512] = cos|cos|sin|sin blocks
    quarter = D_K // 4  # 128
    theta = (1.0 / (10000.0 ** (2.0 * np.arange(quarter, dtype=_F32) / (D_K // 2)))).astype(_F32)
    fx = trk[..., 0:1] * theta                                        # [T, M, 128]
    fy = trk[..., 1:2] * theta
    tct = _bf16(np.concatenate([np.cos(fx), np.cos(fy),
                                np.sin(fx), np.sin(fy)], axis=-1))    # [T, M, 512]

    # Split-precision bias rows: sum_r Bk[r,k] * Bq[r,q] == -16*|t_k - f_q|^2
    # using bf16-exact coarse components so the bias rides a plain bf16 matmul.
    def _split3(x):
        a = (np.round(x * 4.0) / 4.0).astype(_F32)          # 7-bit coarse
        b = (np.round((x - a) * 512.0) / 512.0).astype(_F32)
        l = (x - a - b).astype(_F32)                        # |l| <= 2^-10
        return a, b, l

    def _bfsplit(x64):
        import ml_dtypes
        a = np.asarray(x64, _F32).astype(ml_dtypes.bfloat16).astype(_F32)
        r = x64 - a.astype(np.float64)
        b = r.astype(_F32).astype(ml_dtypes.bfloat16).astype(_F32)
        c = (r - b.astype(np.float64)).astype(_F32)
        return a, b, c

    ta, tb_, tl = _split3(trk)
    fa, fb_, fl = _split3(fp)
    t2a, t2b, t2c = _bfsplit(-16.0 * (trk.astype(np.float64) ** 2).sum(-1))  # [T, M]
    f2a, f2b, f2c = _bfsplit(-16.0 * (fp.astype(np.float64) ** 2).sum(-1))   # [HW]

    onesM = np.ones((T, M), _F32)
    bk = np.stack([32 * ta[..., 0], 32 * ta[..., 1], 32 * ta[..., 0], 32 * ta[..., 1],
                   32 * tb_[..., 0], 32 * tb_[..., 1], 32 * tb_[..., 0], 32 * tb_[..., 1],
                   32 * ta[..., 0], 32 * ta[..., 1], 32 * tl[..., 0], 32 * tl[..., 1],
                   t2a, t2b, t2c, onesM, onesM, onesM], axis=1)       # [T, 18, M]
    onesQ = np.ones(HW, _F32)
    f2aq = np.broadcast_to(f2a, (HW,)); f2bq = np.broadcast_to(f2b, (HW,))
    bq = np.stack([fa[:, 0], fa[:, 1], fb_[:, 0], fb_[:, 1],
                   fa[:, 0], fa[:, 1], fb_[:, 0], fb_[:, 1],
                   fl[:, 0], fl[:, 1], fp[:, 0], fp[:, 1],
                   onesQ, onesQ, onesQ, f2a, f2b, f2c], axis=0)       # [18, HW]
    bk = _bf16(bk)
    bq = _bf16(bq)

    wq = _bf16(np.asarray(W_q, _F32)[:, _PI])
    wk = _bf16(np.asarray(W_k, _F32)[:, _PI])
    wout = _bf16(W_out)
    gam = (np.asarray(q_gamma, _F32) * np.asarray(k_gamma, _F32))[_PI]
    kgam = _bf16(np.broadcast_to(gam, (128, D_K)))
    ident = _bf16(np.eye(128, dtype=_F32))
    ones128 = _bf16(np.ones((128, 128), dtype=_F32))
    return dict(gridT=gridT, uttT=uttT, vro=vro, tct=tct, bk=bk, bq=bq,
                wq=wq, wk=wk, wout=wout, kgam=kgam, ident=ident, ones=ones128)


def _build_nc():
    import concourse.bass as bass
    import concourse.bacc as bacc
    from concourse import mybir
    from concourse import tile
    from contextlib import ExitStack

    f32 = mybir.dt.float32
    f32r = mybir.dt.float32r
    bf16 = mybir.dt.bfloat16
    MUL = mybir.AluOpType.mult
    SUB = mybir.AluOpType.subtract
    ADD = mybir.AluOpType.add
    AF = mybir.ActivationFunctionType

    nc = bacc.Bacc(None, target_bir_lowering=False, debug=False)

    gridT_d = nc.declare_dram_parameter("gridT", [F, D_MODEL, HW], bf16, False)
    uttT_d = nc.declare_dram_parameter("uttT", [F, D_MODEL, M], bf16, False)
    vro_d = nc.declare_dram_parameter("vro", [F, M, D_MODEL], bf16, False)
    tct_d = nc.declare_dram_parameter("tct", [F, M, D_K], bf16, False)
    bk_d = nc.declare_dram_parameter("bk", [F, NB, M], bf16, False)
    bq_d = nc.declare_dram_parameter("bq", [NB, HW], bf16, False)
    wq_d = nc.declare_dram_parameter("wq", [D_MODEL, D_K], bf16, False)
    wk_d = nc.declare_dram_parameter("wk", [D_MODEL, D_K], bf16, False)
    wout_d = nc.declare_dram_parameter("wout", [D_MODEL, D_MODEL], bf16, False)
    kgam_d = nc.declare_dram_parameter("kgam", [128, D_K], bf16, False)
    id_d = nc.declare_dram_parameter("ident", [128, 128], bf16, False)
    ones_d = nc.declare_dram_parameter("ones", [128, 128], bf16, False)
    out_d = nc.declare_dram_parameter("out", [F, HW, D_MODEL], bf16, True)

    with tile.TileContext(nc) as tc_:
        with ExitStack() as stack:
            ep = stack.enter_context
            pconst = ep(tc_.tile_pool(name="pconst", bufs=1))
            pw = ep(tc_.tile_pool(name="pw", bufs=1))
            pwout = ep(tc_.tile_pool(name="pwout", bufs=1))
            pqt = ep(tc_.tile_pool(name="pqt", bufs=1))
            pkt = ep(tc_.tile_pool(name="pkt", bufs=1))
            pv = ep(tc_.tile_pool(name="pv", bufs=1))
            pnt = ep(tc_.tile_pool(name="pnt", bufs=1))
            pbias = ep(tc_.tile_pool(name="pbias", bufs=2))
            ppex = ep(tc_.tile_pool(name="ppex", bufs=8))
            pinu = ep(tc_.tile_pool(name="pinu", bufs=1))
            ping = ep(tc_.tile_pool(name="ping", bufs=2))
            ptrig = ep(tc_.tile_pool(name="ptrig", bufs=4))
            pkro = ep(tc_.tile_pool(name="pkro", bufs=2))
            prt = ep(tc_.tile_pool(name="prt", bufs=1))
            pkn = ep(tc_.tile_pool(name="pkn", bufs=3))
            pst = ep(tc_.tile_pool(name="pst", bufs=4))
            prcp = ep(tc_.tile_pool(name="prcp", bufs=2))
            pf = ep(tc_.tile_pool(name="pf", bufs=6))
            poo = ep(tc_.tile_pool(name="poo", bufs=4))
            ps_acc = ep(tc_.tile_pool(name="ps_acc", bufs=5, space="PSUM"))
            ps_tp = ep(tc_.tile_pool(name="ps_tp", bufs=2, space="PSUM"))
            ps_dn = ep(tc_.tile_pool(name="ps_dn", bufs=1, space="PSUM"))

            ident_s = pconst.tile([128, 128], bf16, tag="ident")
            nc.sync.dma_start(ident_s[:], id_d[:])
            ones_s = pconst.tile([128, 128], bf16, tag="ones")
            nc.sync.dma_start(ones_s[:], ones_d[:])
            kgam_s = pconst.tile([128, D_K], bf16, tag="kgam")
            nc.sync.dma_start(kgam_s[:], kgam_d[:])
            # bias Q-side rows, duplicated at partitions 0 and 64
            bq_s = pconst.tile([128, HW], bf16, tag="bq")
            nc.sync.dma_start(bq_s[0:NB, :], bq_d[:])
            nc.sync.dma_start(bq_s[64:64 + NB, :], bq_d[:])
            eps_s = pconst.tile([128, 1], f32, tag="eps")
            nc.vector.memset(eps_s[:], EPS)

            wk_all = pw.tile([128, 8 * D_K], bf16, tag="wk_all")
            nc.sync.dma_start(wk_all[:].rearrange("p (kc j) -> p kc j", kc=8),
                              wk_d[:].rearrange("(kc p) j -> p kc j", kc=8))
            wq_all = pw.tile([128, 8 * D_K], bf16, tag="wq_all")

            def load_wq():
                nc.sync.dma_start(wq_all[:].rearrange("p (kc j) -> p kc j", kc=8),
                                  wq_d[:].rearrange("(kc p) j -> p kc j", kc=8))
            wq_s = [wq_all[:, kc * D_K:(kc + 1) * D_K] for kc in range(8)]
            wk_s = [wk_all[:, kc * D_K:(kc + 1) * D_K] for kc in range(8)]

            wout_all = pwout.tile([128, 8 * D_MODEL], bf16, tag="wout_all")
            wout_s = {(h, nb): wout_all[:, (2 * h + nb) * 512:(2 * h + nb + 1) * 512]
                      for h in range(8) for nb in range(2)}

            def load_wout():
                nc.sync.dma_start(
                    wout_all[:].rearrange("p (h j) -> p h j", h=8),
                    wout_d[:].rearrange("(h p) j -> p h j", h=8))

            def ln_stats(src_ap):
                """-> (rstd, negmurstd) [128,1] f32 tiles"""
                st6 = pst.tile([128, 6], f32, tag="st6")
                nc.vector.bn_stats(st6[:], src_ap)
                agg = pst.tile([128, 2], f32, tag="agg")
                nc.vector.bn_aggr(agg[:], st6[:])
                sd = pst.tile([128, 1], f32, tag="sd")
                nc.scalar.activation(sd[:], agg[:, 1:2], AF.Sqrt, bias=eps_s[:], scale=1.0)
                rstd = pst.tile([128, 1], f32, tag="rstd")
                nc.vector.reciprocal(rstd[:], sd[:])
                nmr = pst.tile([128, 1], f32, tag="nmr")
                nc.vector.scalar_tensor_tensor(nmr[:], agg[:, 0:1], -1.0, rstd[:],
                                               op0=MUL, op1=MUL)
                return rstd, nmr

            for f in range(F):
                # ---------- per-frame prefetches ----------
                Vro = [pv.tile([128, D_MODEL], bf16, tag=f"V{i}", name=f"V{i}") for i in range(4)]
                trig = []
                for mb in range(4):
                    tt = ptrig.tile([128, 512], bf16, tag="tct", name=f"tct{mb}")
                    nc.sync.dma_start(tt[:], tct_d[f, mb * 128:(mb + 1) * 128, :])
                    trig.append((tt[:, 0:256], tt[:, 256:512]))
                bk_s = pbias.tile([128, M], bf16, tag="bk")
                nc.sync.dma_start(bk_s[0:NB, :], bk_d[f])
                nc.sync.dma_start(bk_s[64:64 + NB, :], bk_d[f])

                # ---------- Phase B: K = gamma' * LN(rope(utt @ Wk)) -> KT2 ----------
                KT2 = [pkt.tile([128, M], bf16, tag=f"KT{i}_{f}", name=f"KT{i}_{f}") for i in range(4)]
                ut_all = pinu.tile([128, 8 * M], bf16, tag="ut")
                nc.sync.dma_start(ut_all[:].rearrange("p (kc m) -> p kc m", kc=8),
                                  uttT_d[f].rearrange("(kc p) m -> p kc m", kc=8))
                if f == 0:
                    load_wq()
                gts = []
                for qh in range(2):
                    g = ping.tile([128, 8 * 512], bf16, tag="gt", name=f"gt{qh}")
                    nc.sync.dma_start(
                        g[:].rearrange("p (kc q) -> p kc q", kc=8),
                        gridT_d[f, :, qh * 512:(qh + 1) * 512].rearrange(
                            "(kc p) q -> p kc q", kc=8))
                    gts.append(g)
                k_ps = {}
                for mbp in range(2):
                    for mb in (2 * mbp, 2 * mbp + 1):
                        k_ps[mb] = ps_acc.tile([128, D_K], f32, tag="acc", name=f"kps{mb}")
                    for kc in range(8):
                        ut = ut_all[:, kc * M:(kc + 1) * M]
                        for mb in (2 * mbp, 2 * mbp + 1):
                            nc.tensor.matmul(k_ps[mb][:], ut[:, mb * 128:(mb + 1) * 128],
                                             wk_s[kc], start=(kc == 0), stop=(kc == 7))
                for mb in range(4):
                    nc.sync.dma_start(Vro[mb][:], vro_d[f, mb * 128:(mb + 1) * 128, :])
                if f == 0:
                    load_wout()
                kf = []
                for mb in range(4):
                    t = pf.tile([128, D_K], f32, tag="pf", name=f"kf{mb}")
                    if mb % 2 == 0:
                        nc.vector.tensor_copy(t[:], k_ps[mb][:])
                    else:
                        nc.scalar.copy(t[:], k_ps[mb][:])
                    kf.append(t)
                for mb in range(4):
                    tcs, tss = trig[mb]
                    kro = pkro.tile([128, D_K], f32, tag="kro")
                    src3 = kf[mb][:].rearrange("p (h t) -> p h t", h=8)
                    dst3 = kro[:].rearrange("p (h t) -> p h t", h=8)
                    x1, x2 = src3[:, :, 0:32], src3[:, :, 32:64]
                    o1, o2 = dst3[:, :, 0:32], dst3[:, :, 32:64]
                    c3 = tcs.rearrange("p (h t) -> p h t", h=8)
                    s3 = tss.rearrange("p (h t) -> p h t", h=8)
                    t1 = prt.tile([128, 256], f32, tag="t1")
                    t2 = prt.tile([128, 256], f32, tag="t2")
                    t3 = prt.tile([128, 256], f32, tag="t3")
                    t4 = prt.tile([128, 256], f32, tag="t4")
                    t13 = t1[:].rearrange("p (h t) -> p h t", h=8)
                    t23 = t2[:].rearrange("p (h t) -> p h t", h=8)
                    t33 = t3[:].rearrange("p (h t) -> p h t", h=8)
                    t43 = t4[:].rearrange("p (h t) -> p h t", h=8)
                    nc.vector.tensor_tensor(t13, x1, c3, op=MUL)
                    nc.vector.tensor_tensor(t23, x2, s3, op=MUL)
                    nc.vector.tensor_tensor(o1, t13, t23, op=SUB)
                    nc.vector.tensor_tensor(t33, x1, s3, op=MUL)
                    nc.vector.tensor_tensor(t43, x2, c3, op=MUL)
                    nc.vector.tensor_tensor(o2, t33, t43, op=ADD)
                    rstd, nmr = ln_stats(kro[:])
                    kn0 = pkn.tile([128, D_K], bf16, tag="kn0")
                    nc.scalar.activation(kn0[:], kro[:], AF.Identity, bias=nmr[:], scale=rstd[:])
                    kn = pkn.tile([128, D_K], bf16, tag="kn")
                    nc.vector.tensor_tensor(kn[:], kn0[:], kgam_s[:], op=MUL)
                    tpt = ps_tp.tile([128, 1024], bf16, tag="tp")
                    for dc in range(4):
                        nc.tensor.transpose(tpt[:, dc * 128:(dc + 1) * 128],
                                            kn[:, dc * 128:(dc + 1) * 128], ident_s[:])
                    for dc in range(4):
                        if dc % 2 == 0:
                            nc.vector.tensor_copy(KT2[dc][:, mb * 128:(mb + 1) * 128],
                                                  tpt[:, dc * 128:(dc + 1) * 128])
                        else:
                            nc.scalar.copy(KT2[dc][:, mb * 128:(mb + 1) * 128],
                                           tpt[:, dc * 128:(dc + 1) * 128])

                # ---------- Phase A: Q = LN(grid @ Wq) -> QT2 ----------
                QT2 = [pqt.tile([128, HW], bf16, tag=f"QT{i}_{f}", name=f"QT{i}_{f}") for i in range(4)]
                for qh in range(2):
                    gt_all = gts[qh]
                    q_ps = {}
                    for qjp in range(2):
                        for qj in (2 * qjp, 2 * qjp + 1):
                            q_ps[qj] = ps_acc.tile([128, D_K], f32, tag="acc", name=f"qps{qj}")
                        for kc in range(8):
                            gt = gt_all[:, kc * 512:(kc + 1) * 512]
                            for qj in (2 * qjp, 2 * qjp + 1):
                                nc.tensor.matmul(q_ps[qj][:], gt[:, qj * 128:(qj + 1) * 128],
                                                 wq_s[kc], start=(kc == 0), stop=(kc == 7))
                    qf = []
                    for qj in range(4):
                        t = pf.tile([128, D_K], f32, tag="pf", name=f"qf{qj}")
                        if qj % 2 == 0:
                            nc.scalar.copy(t[:], q_ps[qj][:])
                        else:
                            nc.vector.tensor_copy(t[:], q_ps[qj][:])
                        qf.append(t)
                    for qj in range(4):
                        qb = qh * 4 + qj
                        rstd, nmr = ln_stats(qf[qj][:])
                        qn = pkn.tile([128, D_K], bf16, tag="qn")
                        nc.scalar.activation(qn[:], qf[qj][:], AF.Identity, bias=nmr[:], scale=rstd[:])
                        tpt = ps_tp.tile([128, 1024], bf16, tag="tp")
                        for dc in range(4):
                            nc.tensor.transpose(tpt[:, dc * 128:(dc + 1) * 128],
                                                qn[:, dc * 128:(dc + 1) * 128], ident_s[:])
                        for dc in range(4):
                            if dc % 2 == 0:
                                nc.scalar.copy(QT2[dc][:, qb * 128:(qb + 1) * 128],
                                               tpt[:, dc * 128:(dc + 1) * 128])
                            else:
                                nc.vector.tensor_copy(QT2[dc][:, qb * 128:(qb + 1) * 128],
                                                      tpt[:, dc * 128:(dc + 1) * 128])

                # ---------- Phase D: per-head attention (bias via f32r row-group mm) ----------
                NT = [pnt.tile([128, HW], bf16, tag=f"NT{i}", name=f"NT{i}") for i in range(8)]
                for h in range(8):
                    dc, sub = divmod(h, 2)
                    po = sub * 64
                    bo = 64 - po  # bias rows live in the other row-group half
                    Pex = [ppex.tile([128, HW], bf16, tag="pex", name=f"pex{i}") for i in range(4)]
                    for mb in range(4):
                        for nb in range(2):
                            sl = slice(nb * 512, (nb + 1) * 512)
                            s_ps = ps_acc.tile([128, 512], f32, tag="acc")
                            nc.tensor.matmul(
                                s_ps[:],
                                bk_s[bo:bo + NB, mb * 128:(mb + 1) * 128],
                                bq_s[bo:bo + NB, sl],
                                start=True, stop=False, tile_position=(bo, 0))
                            nc.tensor.matmul(
                                s_ps[:],
                                KT2[dc][po:po + 64, mb * 128:(mb + 1) * 128],
                                QT2[dc][po:po + 64, sl],
                                start=False, stop=True, tile_position=(po, 0))
                            nc.scalar.activation(Pex[mb][:, sl], s_ps[:],
                                                 AF.Exp, bias=0.0, scale=0.125)
                    for nb in range(2):
                        sl = slice(nb * 512, (nb + 1) * 512)
                        dnb = ps_dn.tile([128, 512], f32, tag="dn")
                        for mb in range(4):
                            nc.tensor.matmul(
                                dnb[:],
                                ones_s[:],
                                Pex[mb][:, sl],
                                start=(mb == 0), stop=(mb == 3))
                        rcp = prcp.tile([128, 512], f32, tag="rcp")
                        nc.vector.reciprocal_approx_fast(rcp[:], dnb[:])
                        nm_ps = ps_acc.tile([128, 512], f32, tag="acc")
                        for mb in range(4):
                            nc.tensor.matmul(
                                nm_ps[:],
                                Vro[mb][:, h * 128:(h + 1) * 128],
                                Pex[mb][:, sl],
                                start=(mb == 0), stop=(mb == 3))
                        nc.vector.tensor_tensor(
                            NT[h][:, sl], nm_ps[:], rcp[:], op=MUL)

                # ---------- Phase E: out = sampled @ Wout ----------
                for qb in range(8):
                    oo = poo.tile([128, 1024], bf16, tag="oo")
                    for nb in range(2):
                        o_ps = ps_acc.tile([128, 512], f32, tag="acc")
                        for h in range(8):
                            nc.tensor.matmul(
                                o_ps[:],
                                NT[h][:, qb * 128:(qb + 1) * 128],
                                wout_s[(h, nb)],
                                start=(h == 0), stop=(h == 7))
                        if nb == 0:
                            nc.vector.tensor_copy(oo[:, 0:512], o_ps[:])
                        else:
                            nc.scalar.copy(oo[:, 512:1024], o_ps[:])
                    nc.sync.dma_start(out_d[f, qb * 128:(qb + 1) * 128, :], oo[:])

    nc.compile()
    return nc


_NC_CACHE = None
_LAST = None


def _get_nc():
    global _NC_CACHE
    if _NC_CACHE is None:
        _NC_CACHE = _build_nc()
    return _NC_CACHE


def kernel(**inputs) -> np.ndarray:
    global _LAST
    prep = _host_prep(**inputs)
    from concourse.bass_utils import run_bass_kernel_spmd
    nc = _get_nc()
    in_maps = []
    for c in range(NCORES):
        sl = slice(c * F, (c + 1) * F)
        in_maps.append({
            "gridT": prep["gridT"][sl], "uttT": prep["uttT"][sl],
            "vro": prep["vro"][sl], "tct": prep["tct"][sl],
            "bk": prep["bk"][sl], "bq": prep["bq"],
            "wq": prep["wq"], "wk": prep["wk"], "wout": prep["wout"],
            "kgam": prep["kgam"], "ident": prep["ident"], "ones": prep["ones"],
        })
    trace = bool(os.environ.get("KBENCH_TRACE"))
    res = run_bass_kernel_spmd(nc, in_maps, core_ids=list(range(NCORES)),
                               trace=trace)
    _LAST = res
    out = np.concatenate([np.asarray(res.results[c]["out"]).astype(np.float32)
                          for c in range(NCORES)], axis=0)
    return np.ascontiguousarray(out.reshape(T, HW, D_MODEL))


# revision 18
# speedup vs baseline: 1.2278x; 1.1101x over previous
import sys, os
sys.path.insert(0, "/opt/trn_rl_repo")
sys.path.insert(0, "/opt/trn_rl_repo/concourse")

import numpy as np

T, HW, M = 16, 1024, 512
D_MODEL, D_K, H = 1024, 512, 8
HD = D_K // H      # 64
VD = D_MODEL // H  # 128
SIGMA = 0.5
EPS = 1e-6
NCORES = 8
F = T // NCORES    # frames per core = 2
NB = 18            # split-precision bias rows (bf16-exact 3-level split)

_F32 = np.float32

# per-head rope de-interleave permutation of the D_K dims:
# head h keeps cols [64h,64h+64); within a head evens first then odds.
_PI = np.empty(D_K, np.int64)
for _h in range(H):
    for _i in range(32):
        _PI[64 * _h + _i] = 64 * _h + 2 * _i
        _PI[64 * _h + 32 + _i] = 64 * _h + 2 * _i + 1


def _bf16(x):
    import ml_dtypes
    return np.asarray(x, np.float32).astype(ml_dtypes.bfloat16)


def _r11(x):
    """Round fp32 to 10 explicit mantissa bits (f32r/tf32-representable)."""
    b = np.asarray(x, np.float32).view(np.uint32)
    b = (b + 0x1000) & np.uint32(0xFFFFE000)
    return b.view(np.float32)


def _host_prep(updated_track_tokens, tracks, feature_positions, original_features,
               grid_coords_tokens, W_q, W_k, W_out, q_gamma, k_gamma):
    utt = np.ascontiguousarray(updated_track_tokens, dtype=_F32)      # [T, M, D]
    grid = np.ascontiguousarray(grid_coords_tokens, dtype=_F32)       # [T, HW, D]
    trk = np.ascontiguousarray(tracks, dtype=_F32)                    # [T, M, 2]
    fp = np.ascontiguousarray(feature_positions, dtype=_F32)          # [HW, 2]

    gridT = _bf16(grid.transpose(0, 2, 1))                            # [T, D, HW]
    uttT = _bf16(utt.transpose(0, 2, 1))                              # [T, D, M]

    # V = rope_2d(utt, tracks) computed on host (elementwise table math)
    def rope_2d(x, pos):
        B, N, D = x.shape
        half, quarter = D // 2, D // 4
        theta = 1.0 / (10000.0 ** (2.0 * np.arange(quarter, dtype=_F32) / half))
        fx = pos[..., 0:1] * theta
        fy = pos[..., 1:2] * theta

        def rot(part, f):
            c, s = np.cos(f).astype(_F32), np.sin(f).astype(_F32)
            p = part.reshape(B, N, quarter, 2)
            x1, x2 = p[..., 0], p[..., 1]
            return np.stack([x1 * c - x2 * s, x1 * s + x2 * c], -1).reshape(B, N, half)

        return np.concatenate([rot(x[..., :half], fx), rot(x[..., half:], fy)], -1)

    vro = _bf16(rope_2d(utt, trk))                                    # [T, M, D]

    # K-rope trig tables matching pi layout: [T, M, 512] = cos|cos|sin|sin blocks
    quarter = D_K // 4  # 128
    theta = (1.0 / (10000.0 ** (2.0 * np.arange(quarter, dtype=_F32) / (D_K // 2)))).astype(_F32)
    fx = trk[..., 0:1] * theta                                        # [T, M, 128]
    fy = trk[..., 1:2] * theta
    tct = _bf16(np.concatenate([np.cos(fx), np.cos(fy),
                                np.sin(fx), np.sin(fy)], axis=-1))    # [T, M, 512]

    # splat bias as multiplicative table: EB = exp(-2*min(d2,30)), [T, M, HW]
    d2 = ((trk[:, :, None, :] - fp[None, None, :, :]) ** 2).sum(-1)   # [T, M, HW]
    eb = _bf16(np.exp(-2.0 * np.minimum(d2.astype(np.float64), 30.0)))

    wq = _bf16(np.asarray(W_q, _F32)[:, _PI])
    wk = _bf16(np.asarray(W_k, _F32)[:, _PI])
    wout = _bf16(W_out)
    gam = (np.asarray(q_gamma, _F32) * np.asarray(k_gamma, _F32))[_PI]
    kgam = _bf16(np.broadcast_to(gam, (128, D_K)))
    ident = _bf16(np.eye(128, dtype=_F32))
    ones128 = _bf16(np.ones((128, 128), dtype=_F32))
    return dict(gridT=gridT, uttT=uttT, vro=vro, tct=tct, eb=eb,
                wq=wq, wk=wk, wout=wout, kgam=kgam, ident=ident, ones=ones128)


def _build_nc():
    import concourse.bass as bass
    import concourse.bacc as bacc
    from concourse import mybir
    from concourse import tile
    from contextlib import ExitStack

    f32 = mybir.dt.float32
    f32r = mybir.dt.float32r
    bf16 = mybir.dt.bfloat16
    MUL = mybir.AluOpType.mult
    SUB = mybir.AluOpType.subtract
    ADD = mybir.AluOpType.add
    AF = mybir.ActivationFunctionType

    nc = bacc.Bacc(None, target_bir_lowering=False, debug=False)

    gridT_d = nc.declare_dram_parameter("gridT", [F, D_MODEL, HW], bf16, False)
    uttT_d = nc.declare_dram_parameter("uttT", [F, D_MODEL, M], bf16, False)
    vro_d = nc.declare_dram_parameter("vro", [F, M, D_MODEL], bf16, False)
    tct_d = nc.declare_dram_parameter("tct", [F, M, D_K], bf16, False)
    eb_d = nc.declare_dram_parameter("eb", [F, M, HW], bf16, False)
    wq_d = nc.declare_dram_parameter("wq", [D_MODEL, D_K], bf16, False)
    wk_d = nc.declare_dram_parameter("wk", [D_MODEL, D_K], bf16, False)
    wout_d = nc.declare_dram_parameter("wout", [D_MODEL, D_MODEL], bf16, False)
    kgam_d = nc.declare_dram_parameter("kgam", [128, D_K], bf16, False)
    id_d = nc.declare_dram_parameter("ident", [128, 128], bf16, False)
    ones_d = nc.declare_dram_parameter("ones", [128, 128], bf16, False)
    out_d = nc.declare_dram_parameter("out", [F, HW, D_MODEL], bf16, True)

    with tile.TileContext(nc) as tc_:
        with ExitStack() as stack:
            ep = stack.enter_context
            pconst = ep(tc_.tile_pool(name="pconst", bufs=1))
            pw = ep(tc_.tile_pool(name="pw", bufs=1))
            pwout = ep(tc_.tile_pool(name="pwout", bufs=1))
            pqt = ep(tc_.tile_pool(name="pqt", bufs=1))
            pkt = ep(tc_.tile_pool(name="pkt", bufs=1))
            pv = ep(tc_.tile_pool(name="pv", bufs=1))
            pnt = ep(tc_.tile_pool(name="pnt", bufs=1))
            peb = ep(tc_.tile_pool(name="peb", bufs=1))
            ppraw = ep(tc_.tile_pool(name="ppraw", bufs=6))
            ppex = ep(tc_.tile_pool(name="ppex", bufs=16))
            pinu = ep(tc_.tile_pool(name="pinu", bufs=1))
            ping = ep(tc_.tile_pool(name="ping", bufs=2))
            ptrig = ep(tc_.tile_pool(name="ptrig", bufs=4))
            pkro = ep(tc_.tile_pool(name="pkro", bufs=2))
            prt = ep(tc_.tile_pool(name="prt", bufs=1))
            pkn = ep(tc_.tile_pool(name="pkn", bufs=3))
            pst = ep(tc_.tile_pool(name="pst", bufs=4))
            prcp = ep(tc_.tile_pool(name="prcp", bufs=2))
            pf = ep(tc_.tile_pool(name="pf", bufs=6))
            poo = ep(tc_.tile_pool(name="poo", bufs=4))
            ps_acc = ep(tc_.tile_pool(name="ps_acc", bufs=5, space="PSUM"))
            ps_tp = ep(tc_.tile_pool(name="ps_tp", bufs=2, space="PSUM"))
            ps_dn = ep(tc_.tile_pool(name="ps_dn", bufs=1, space="PSUM"))

            ident_s = pconst.tile([128, 128], bf16, tag="ident")
            nc.sync.dma_start(ident_s[:], id_d[:])
            ones_s = pconst.tile([128, 128], bf16, tag="ones")
            nc.sync.dma_start(ones_s[:], ones_d[:])
            kgam_s = pconst.tile([128, D_K], bf16, tag="kgam")
            nc.sync.dma_start(kgam_s[:], kgam_d[:])
            eps_s = pconst.tile([128, 1], f32, tag="eps")
            nc.vector.memset(eps_s[:], EPS)

            wk_all = pw.tile([128, 8 * D_K], bf16, tag="wk_all")
            nc.sync.dma_start(wk_all[:].rearrange("p (kc j) -> p kc j", kc=8),
                              wk_d[:].rearrange("(kc p) j -> p kc j", kc=8))
            wq_all = pw.tile([128, 8 * D_K], bf16, tag="wq_all")

            def load_wq():
                nc.sync.dma_start(wq_all[:].rearrange("p (kc j) -> p kc j", kc=8),
                                  wq_d[:].rearrange("(kc p) j -> p kc j", kc=8))
            wq_s = [wq_all[:, kc * D_K:(kc + 1) * D_K] for kc in range(8)]
            wk_s = [wk_all[:, kc * D_K:(kc + 1) * D_K] for kc in range(8)]

            wout_all = pwout.tile([128, 8 * D_MODEL], bf16, tag="wout_all")
            wout_s = {(h, nb): wout_all[:, (2 * h + nb) * 512:(2 * h + nb + 1) * 512]
                      for h in range(8) for nb in range(2)}

            def load_wout():
                nc.sync.dma_start(
                    wout_all[:].rearrange("p (h j) -> p h j", h=8),
                    wout_d[:].rearrange("(h p) j -> p h j", h=8))

            def ln_stats(src_ap):
                """-> (rstd, negmurstd) [128,1] f32 tiles"""
                st6 = pst.tile([128, 6], f32, tag="st6")
                nc.vector.bn_stats(st6[:], src_ap)
                agg = pst.tile([128, 2], f32, tag="agg")
                nc.vector.bn_aggr(agg[:], st6[:])
                sd = pst.tile([128, 1], f32, tag="sd")
                nc.scalar.activation(sd[:], agg[:, 1:2], AF.Sqrt, bias=eps_s[:], scale=1.0)
                rstd = pst.tile([128, 1], f32, tag="rstd")
                nc.vector.reciprocal(rstd[:], sd[:])
                nmr = pst.tile([128, 1], f32, tag="nmr")
                nc.vector.scalar_tensor_tensor(nmr[:], agg[:, 0:1], -1.0, rstd[:],
                                               op0=MUL, op1=MUL)
                return rstd, nmr

            pending_E = []
            for f in range(F):
                # ---------- per-frame prefetches ----------
                Vro = [pv.tile([128, D_MODEL], bf16, tag=f"V{i}", name=f"V{i}") for i in range(4)]
                trig = []
                for mb in range(4):
                    tt = ptrig.tile([128, 512], bf16, tag="tct", name=f"tct{mb}")
                    nc.sync.dma_start(tt[:], tct_d[f, mb * 128:(mb + 1) * 128, :])
                    trig.append((tt[:, 0:256], tt[:, 256:512]))
                EBs = [peb.tile([128, HW], bf16, tag=f"EB{i}", name=f"EB{i}") for i in range(4)]

                # ---------- Phase B: K = gamma' * LN(rope(utt @ Wk)) -> KT2 ----------
                KT2 = [pkt.tile([128, M], bf16, tag=f"KT{i}_{f}", name=f"KT{i}_{f}") for i in range(4)]
                ut_all = pinu.tile([128, 8 * M], bf16, tag="ut")
                nc.sync.dma_start(ut_all[:].rearrange("p (kc m) -> p kc m", kc=8),
                                  uttT_d[f].rearrange("(kc p) m -> p kc m", kc=8))
                if f == 0:
                    load_wq()
                gts = []
                for qh in range(2):
                    g = ping.tile([128, 8 * 512], bf16, tag="gt", name=f"gt{qh}")
                    nc.sync.dma_start(
                        g[:].rearrange("p (kc q) -> p kc q", kc=8),
                        gridT_d[f, :, qh * 512:(qh + 1) * 512].rearrange(
                            "(kc p) q -> p kc q", kc=8))
                    gts.append(g)
                k_ps = {}
                for mbp in range(2):
                    for mb in (2 * mbp, 2 * mbp + 1):
                        k_ps[mb] = ps_acc.tile([128, D_K], f32, tag="acc", name=f"kps{mb}")
                    for kc in range(8):
                        ut = ut_all[:, kc * M:(kc + 1) * M]
                        for mb in (2 * mbp, 2 * mbp + 1):
                            nc.tensor.matmul(k_ps[mb][:], ut[:, mb * 128:(mb + 1) * 128],
                                             wk_s[kc], start=(kc == 0), stop=(kc == 7))
                for mb in range(4):
                    nc.sync.dma_start(EBs[mb][:], eb_d[f, mb * 128:(mb + 1) * 128, :])
                    nc.sync.dma_start(Vro[mb][:], vro_d[f, mb * 128:(mb + 1) * 128, :])
                if f == 0:
                    load_wout()
                kf = []
                for mb in range(4):
                    t = pf.tile([128, D_K], f32, tag="pf", name=f"kf{mb}")
                    if mb % 2 == 0:
                        nc.vector.tensor_copy(t[:], k_ps[mb][:])
                    else:
                        nc.scalar.copy(t[:], k_ps[mb][:])
                    kf.append(t)
                for mb in range(4):
                    tcs, tss = trig[mb]
                    kro = pkro.tile([128, D_K], f32, tag="kro")
                    src3 = kf[mb][:].rearrange("p (h t) -> p h t", h=8)
                    dst3 = kro[:].rearrange("p (h t) -> p h t", h=8)
                    x1, x2 = src3[:, :, 0:32], src3[:, :, 32:64]
                    o1, o2 = dst3[:, :, 0:32], dst3[:, :, 32:64]
                    c3 = tcs.rearrange("p (h t) -> p h t", h=8)
                    s3 = tss.rearrange("p (h t) -> p h t", h=8)
                    t1 = prt.tile([128, 256], f32, tag="t1")
                    t2 = prt.tile([128, 256], f32, tag="t2")
                    t3 = prt.tile([128, 256], f32, tag="t3")
                    t4 = prt.tile([128, 256], f32, tag="t4")
                    t13 = t1[:].rearrange("p (h t) -> p h t", h=8)
                    t23 = t2[:].rearrange("p (h t) -> p h t", h=8)
                    t33 = t3[:].rearrange("p (h t) -> p h t", h=8)
                    t43 = t4[:].rearrange("p (h t) -> p h t", h=8)
                    nc.vector.tensor_tensor(t13, x1, c3, op=MUL)
                    nc.vector.tensor_tensor(t23, x2, s3, op=MUL)
                    nc.vector.tensor_tensor(o1, t13, t23, op=SUB)
                    nc.vector.tensor_tensor(t33, x1, s3, op=MUL)
                    nc.vector.tensor_tensor(t43, x2, c3, op=MUL)
                    nc.vector.tensor_tensor(o2, t33, t43, op=ADD)
                    rstd, nmr = ln_stats(kro[:])
                    kn0 = pkn.tile([128, D_K], bf16, tag="kn0")
                    nc.scalar.activation(kn0[:], kro[:], AF.Identity, bias=nmr[:], scale=rstd[:])
                    kn = pkn.tile([128, D_K], bf16, tag="kn")
                    nc.vector.tensor_tensor(kn[:], kn0[:], kgam_s[:], op=MUL)
                    tpt = ps_tp.tile([128, 1024], bf16, tag="tp")
                    for dc in range(4):
                        nc.tensor.transpose(tpt[:, dc * 128:(dc + 1) * 128],
                                            kn[:, dc * 128:(dc + 1) * 128], ident_s[:])
                    for dc in range(4):
                        if dc % 2 == 0:
                            nc.vector.tensor_copy(KT2[dc][:, mb * 128:(mb + 1) * 128],
                                                  tpt[:, dc * 128:(dc + 1) * 128])
                        else:
                            nc.scalar.copy(KT2[dc][:, mb * 128:(mb + 1) * 128],
                                           tpt[:, dc * 128:(dc + 1) * 128])

                # ---------- Phase A: Q = LN(grid @ Wq) -> QT2 ----------
                QT2 = [pqt.tile([128, HW], bf16, tag=f"QT{i}_{f}", name=f"QT{i}_{f}") for i in range(4)]
                for qh in range(2):
                    gt_all = gts[qh]
                    q_ps = {}
                    for qjp in range(2):
                        for qj in (2 * qjp, 2 * qjp + 1):
                            q_ps[qj] = ps_acc.tile([128, D_K], f32, tag="acc", name=f"qps{qj}")
                        for kc in range(8):
                            gt = gt_all[:, kc * 512:(kc + 1) * 512]
                            for qj in (2 * qjp, 2 * qjp + 1):
                                nc.tensor.matmul(q_ps[qj][:], gt[:, qj * 128:(qj + 1) * 128],
                                                 wq_s[kc], start=(kc == 0), stop=(kc == 7))
                    qf = []
                    for qj in range(4):
                        t = pf.tile([128, D_K], f32, tag="pf", name=f"qf{qj}")
                        if qj % 2 == 0:
                            nc.scalar.copy(t[:], q_ps[qj][:])
                        else:
                            nc.vector.tensor_copy(t[:], q_ps[qj][:])
                        qf.append(t)
                    for qj in range(4):
                        qb = qh * 4 + qj
                        rstd, nmr = ln_stats(qf[qj][:])
                        qn = pkn.tile([128, D_K], bf16, tag="qn")
                        nc.scalar.activation(qn[:], qf[qj][:], AF.Identity, bias=nmr[:], scale=rstd[:])
                        tpt = ps_tp.tile([128, 1024], bf16, tag="tp")
                        for dc in range(4):
                            nc.tensor.transpose(tpt[:, dc * 128:(dc + 1) * 128],
                                                qn[:, dc * 128:(dc + 1) * 128], ident_s[:])
                        for dc in range(4):
                            if dc % 2 == 0:
                                nc.scalar.copy(QT2[dc][:, qb * 128:(qb + 1) * 128],
                                               tpt[:, dc * 128:(dc + 1) * 128])
                            else:
                                nc.vector.tensor_copy(QT2[dc][:, qb * 128:(qb + 1) * 128],
                                                      tpt[:, dc * 128:(dc + 1) * 128])

                while len(pending_E) > 0:
                    pending_E.pop(0)()

                # ---------- Phase D: SW-pipelined per-head attention ----------
                NT = [pnt.tile([128, HW], bf16, tag=f"NT{i}", name=f"NT{i}") for i in range(8)]
                PexAll = {}

                def emit_scores(h):
                    dc, sub = divmod(h, 2)
                    po = sub * 64
                    Pex = [ppex.tile([128, HW], bf16, tag="pex", name=f"pex{h%3}_{i}")
                           for i in range(4)]
                    PexAll[h] = Pex
                    for mb in range(4):
                        praw = ppraw.tile([128, HW], bf16, tag="praw", name=f"praw{h%3}")
                        for nb in range(2):
                            sl = slice(nb * 512, (nb + 1) * 512)
                            s_ps = ps_acc.tile([128, 512], f32, tag="acc")
                            nc.tensor.matmul(
                                s_ps[:],
                                KT2[dc][po:po + 64, mb * 128:(mb + 1) * 128],
                                QT2[dc][po:po + 64, sl],
                                start=True, stop=True)
                            nc.scalar.activation(praw[:, sl], s_ps[:],
                                                 AF.Exp, bias=0.0, scale=0.125)
                        nc.gpsimd.tensor_mul(Pex[mb][:], praw[:], EBs[mb][:])

                def emit_reduce(h):
                    Pex = PexAll.pop(h)
                    for nb in range(2):
                        sl = slice(nb * 512, (nb + 1) * 512)
                        dnb = ps_dn.tile([128, 512], f32, tag="dn")
                        for mb in range(4):
                            nc.tensor.matmul(
                                dnb[:],
                                ones_s[:],
                                Pex[mb][:, sl],
                                start=(mb == 0), stop=(mb == 3))
                        rcp = prcp.tile([128, 512], f32, tag="rcp")
                        nc.vector.reciprocal_approx_fast(rcp[:], dnb[:])
                        nm_ps = ps_acc.tile([128, 512], f32, tag="acc")
                        for mb in range(4):
                            nc.tensor.matmul(
                                nm_ps[:],
                                Vro[mb][:, h * 128:(h + 1) * 128],
                                Pex[mb][:, sl],
                                start=(mb == 0), stop=(mb == 3))
                        nc.vector.tensor_tensor(
                            NT[h][:, sl], nm_ps[:], rcp[:], op=MUL)

                for h in range(8):
                    emit_scores(h)
                    if h >= 2:
                        emit_reduce(h - 2)
                emit_reduce(6)
                emit_reduce(7)

                # ---------- Phase E (deferred): out = sampled @ Wout ----------
                def emit_E(f=f, NT=NT):
                    for qb in range(8):
                        oo = poo.tile([128, 1024], bf16, tag="oo", name=f"oo{qb%4}")
                        for nb in range(2):
                            o_ps = ps_acc.tile([128, 512], f32, tag="acc")
                            for h in range(8):
                                nc.tensor.matmul(
                                    o_ps[:],
                                    NT[h][:, qb * 128:(qb + 1) * 128],
                                    wout_s[(h, nb)],
                                    start=(h == 0), stop=(h == 7))
                            if nb == 0:
                                nc.vector.tensor_copy(oo[:, 0:512], o_ps[:])
                            else:
                                nc.scalar.copy(oo[:, 512:1024], o_ps[:])
                        nc.sync.dma_start(out_d[f, qb * 128:(qb + 1) * 128, :], oo[:])
                pending_E.append(emit_E)

            while len(pending_E) > 0:
                pending_E.pop(0)()

    nc.compile()
    return nc


_NC_CACHE = None
_LAST = None


def _get_nc():
    global _NC_CACHE
    if _NC_CACHE is None:
        _NC_CACHE = _build_nc()
    return _NC_CACHE


def kernel(**inputs) -> np.ndarray:
    global _LAST
    prep = _host_prep(**inputs)
    from concourse.bass_utils import run_bass_kernel_spmd
    nc = _get_nc()
    in_maps = []
    for c in range(NCORES):
        sl = slice(c * F, (c + 1) * F)
        in_maps.append({
            "gridT": prep["gridT"][sl], "uttT": prep["uttT"][sl],
            "vro": prep["vro"][sl], "tct": prep["tct"][sl],
            "eb": prep["eb"][sl],
            "wq": prep["wq"], "wk": prep["wk"], "wout": prep["wout"],
            "kgam": prep["kgam"], "ident": prep["ident"], "ones": prep["ones"],
        })
    trace = bool(os.environ.get("KBENCH_TRACE"))
    res = run_bass_kernel_spmd(nc, in_maps, core_ids=list(range(NCORES)),
                               trace=trace)
    _LAST = res
    out = np.concatenate([np.asarray(res.results[c]["out"]).astype(np.float32)
                          for c in range(NCORES)], axis=0)
    return np.ascontiguousarray(out.reshape(T, HW, D_MODEL))
